# revision 5
# baseline (speedup 1.0000x reference)
"""Trainium2 Bass kernel for the NeuromorphicNetwork (LIF spiking net) problem.

Computation (see problem reference):
  rates = sigmoid(x @ enc_W.T + enc_b)            [B, D]
  100 steps of 3-layer LIF:
    v_L = v_L*decay + (in_L @ W_L)*(1-decay);  s_L = v_L >= 1;  v_L = v_L*(v_L<1)
  out = (sum_t s_3) / 100                          [B, A]

Strategy:
  - Data-parallel over batch: 8192 rows -> 8 cores x 1024.
  - Feature-major on-chip layout: activations stored [features, batch] so the
    recurrent matmuls need no transposes (weights stationary, batch streams).
  - rates@W1 is constant across the scan: precomputed once on-device (fp32
    matmuls), so layer 1 needs no per-step matmul at all.
  - Layer 2/3 synaptic matmuls: weights pre-split on host into 3 bf16
    components (hi/lo/lo2, ~24 mantissa bits total). Spikes are {0,1} which
    is exact in bf16, so each product is exact and the fp32 PSUM sum matches
    an fp32 matmul to ~1 ulp -- at 1 bf16 cycle/column instead of 4 for fp32.
  - LIF pointwise update per tile is 3 fused ops:
      v' = (u * decay) + syn       (scalar_tensor_tensor, reads PSUM)
      s  = (v' >= 1)               (tensor_scalar -> bf16 {0,1})
      u  = (v' < 1) * v'           (scalar_tensor_tensor, in-place)
    split across the Vector and GpSimd engines.
  - Output spike accumulation: identity-matmul of s3 into a persistent PSUM
    bank (exact integer counts); /100 happens on host in fp32 (bit-identical
    to the reference's acc/SIM_TIME).
"""

import numpy as np
import ml_dtypes

import bass_rust
import concourse.bass as bass
import concourse.tile as tile
from concourse import mybir
from concourse.bass_utils import run_bass_kernel_spmd

F32 = mybir.dt.float32
BF16 = mybir.dt.bfloat16
AOP = mybir.AluOpType
AF = mybir.ActivationFunctionType

TAU = 20.0
THRESHOLD = 1.0
DT = 1.0
SIM_TIME = 100
B, D, H, A = 8192, 256, 256, 64
N_CORES = 8
BC = B // N_CORES          # batch per core (1024)
CHUNK = 512                # batch chunk for pipelining
NCHUNK = BC // CHUNK
DECAY = float(np.exp(np.float32(-DT / TAU)).astype(np.float32))
ONE_MINUS_DECAY = float(np.float32(1.0) - np.float32(DECAY))
NSPLIT = 3                 # bf16 weight splits


def _split_multiwaits(nc):
    """walrus in this toolchain crashes on instructions carrying >=2 sem
    waits; split extras onto single-wait NoOps just before the instruction."""
    f = nc.m.functions[0]
    for bb in f.blocks:
        insts = list(bb.instructions)
        changed = False
        new_list = []
        for inst in insts:
            si = inst.sync_info
            if si is not None and si.on_wait is not None and len(si.on_wait) > 1:
                waits = list(si.on_wait)
                for w in waits[:-1]:
                    nop = mybir.InstNoOp(name=f"{inst.name}-sw-{w.id}", ins=[], outs=[])
                    nop.engine = inst.engine
                    nop.sync_info = bass_rust.SyncInfo(on_wait=[w], on_update=[])
                    new_list.append(nop)
                inst.sync_info = bass_rust.SyncInfo(
                    on_wait=[waits[-1]], on_update=list(si.on_update or [])
                )
                changed = True
            new_list.append(inst)
        if changed:
            bb.instructions = new_list


def _bf16_split(w, n=NSPLIT):
    """Split fp32 array into n bf16 arrays summing to ~w (24+ bits for n=3)."""
    parts = []
    resid = w.astype(np.float32)
    for _ in range(n):
        p = resid.astype(ml_dtypes.bfloat16)
        parts.append(p)
        resid = resid - p.astype(np.float32)
    return parts


def _build_program(n_steps):
    nc = bass.Bass(trn_type="TRN2", target_bir_lowering=False, debug=False)

    # ---- DRAM I/O (per core) ----
    xT_d = nc.dram_tensor("xT", [D, BC], F32, kind="ExternalInput").ap()
    encWt_d = nc.dram_tensor("encWt", [D, D], F32, kind="ExternalInput").ap()
    encb_d = nc.dram_tensor("encb", [D, 1], F32, kind="ExternalInput").ap()
    w1_d = nc.dram_tensor("w1", [D, H], F32, kind="ExternalInput").ap()
    w2s_d = nc.dram_tensor("w2s", [NSPLIT, H, H], BF16, kind="ExternalInput").ap()
    w3s_d = nc.dram_tensor("w3s", [NSPLIT, H, A], BF16, kind="ExternalInput").ap()
    eyeA_d = nc.dram_tensor("eyeA", [A, A], BF16, kind="ExternalInput").ap()
    acc_d = nc.dram_tensor("acc", [A, BC], F32, kind="ExternalOutput").ap()

    KT_D = D // 128  # k-tiles over 256 features
    MT_H = H // 128  # m-tiles of hidden

    with tile.TileContext(nc) as tc:
        with (
            tc.tile_pool(name="persist", bufs=1) as pp,
            tc.tile_pool(name="spikes", bufs=3) as sp,
            tc.tile_pool(name="pre", bufs=1) as pre,
            tc.tile_pool(name="psyn", bufs=3, space="PSUM") as psyn,
            tc.tile_pool(name="psyn3", bufs=2, space="PSUM") as psyn3,
            tc.tile_pool(name="pacc", bufs=1, space="PSUM") as pacc,
        ):
            # ---------- one-time: load + precompute ----------
            xT = [pre.tile([128, BC], F32, name=f"xT{k}") for k in range(KT_D)]
            encW = [
                [pre.tile([128, 128], F32, name=f"encW{k}{m}") for m in range(KT_D)]
                for k in range(KT_D)
            ]
            encb = [pre.tile([128, 1], F32, name=f"encb{m}") for m in range(MT_H)]
            w1 = [
                [pre.tile([128, 128], F32, name=f"w1{k}{m}") for m in range(MT_H)]
                for k in range(KT_D)
            ]
            for k in range(KT_D):
                nc.gpsimd.dma_start(out=xT[k][:], in_=xT_d[128 * k : 128 * (k + 1), :])
                for m in range(KT_D):
                    nc.gpsimd.dma_start(
                        out=encW[k][m][:],
                        in_=encWt_d[128 * k : 128 * (k + 1), 128 * m : 128 * (m + 1)],
                    )
                    nc.gpsimd.dma_start(
                        out=w1[k][m][:],
                        in_=w1_d[128 * k : 128 * (k + 1), 128 * m : 128 * (m + 1)],
                    )
            for m in range(MT_H):
                nc.gpsimd.dma_start(out=encb[m][:], in_=encb_d[128 * m : 128 * (m + 1), :])

            # weights for layers 2/3 (bf16 splits) + identity for acc
            w2 = [
                [
                    [pre.tile([128, 128], BF16, name=f"w2_{s}_{k}{m}") for m in range(MT_H)]
                    for k in range(MT_H)
                ]
                for s in range(NSPLIT)
            ]
            w3 = [
                [pre.tile([128, A], BF16, name=f"w3_{s}_{k}") for k in range(MT_H)]
                for s in range(NSPLIT)
            ]
            eyeA = pre.tile([A, A], BF16, name="eyeA")
            nc.gpsimd.dma_start(out=eyeA[:], in_=eyeA_d[:, :])
            for s in range(NSPLIT):
                for k in range(MT_H):
                    nc.gpsimd.dma_start(
                        out=w3[s][k][:], in_=w3s_d[s, 128 * k : 128 * (k + 1), :]
                    )
                    for m in range(MT_H):
                        nc.gpsimd.dma_start(
                            out=w2[s][k][m][:],
                            in_=w2s_d[s, 128 * k : 128 * (k + 1), 128 * m : 128 * (m + 1)],
                        )

            # rates = sigmoid(encW @ xT + b); P1 = (rates @ W1) * (1-decay)
            rates = [pre.tile([128, BC], F32, name=f"rates{m}") for m in range(MT_H)]
            P1 = [pre.tile([128, BC], F32, name=f"P1{m}") for m in range(MT_H)]
            for m in range(MT_H):
                for c in range(NCHUNK):
                    pt = psyn.tile([128, CHUNK], F32, tag="pv", name=f"pt_{m}_{c}")
                    for k in range(KT_D):
                        nc.tensor.matmul(
                            pt[:],
                            encW[k][m][:],
                            xT[k][:, c * CHUNK : (c + 1) * CHUNK],
                            start=(k == 0),
                            stop=(k == KT_D - 1),
                        )
                    nc.scalar.activation(
                        rates[m][:, c * CHUNK : (c + 1) * CHUNK],
                        pt[:],
                        AF.Sigmoid,
                        bias=encb[m][:],
                        scale=1.0,
                    )
            for m in range(MT_H):
                for c in range(NCHUNK):
                    pt = psyn.tile([128, CHUNK], F32, tag="pv", name=f"pt_{m}_{c}")
                    for k in range(KT_D):
                        nc.tensor.matmul(
                            pt[:],
                            w1[k][m][:],
                            rates[k][:, c * CHUNK : (c + 1) * CHUNK],
                            start=(k == 0),
                            stop=(k == KT_D - 1),
                        )
                    # P1 = psum * (1-decay), matching reference rounding
                    nc.scalar.activation(
                        P1[m][:, c * CHUNK : (c + 1) * CHUNK],
                        pt[:],
                        AF.Copy,
                        scale=ONE_MINUS_DECAY,
                    )

            # ---------- persistent state ----------
            u1 = [pp.tile([128, BC], F32, name=f"u1_{m}") for m in range(MT_H)]
            u2 = [pp.tile([128, BC], F32, name=f"u2_{m}") for m in range(MT_H)]
            u3 = pp.tile([A, BC], F32, name="u3")
            for m in range(MT_H):
                nc.vector.memset(u1[m][:], 0.0)
                nc.vector.memset(u2[m][:], 0.0)
            nc.vector.memset(u3[:], 0.0)
            acc = pacc.tile([A, BC], F32, name="accp")

            # ---------- the 100-step scan ----------
            for t in range(n_steps):
                s1 = [sp.tile([128, BC], BF16, tag=f"s1_{m}", name=f"s1_{m}_{t}") for m in range(MT_H)]
                s2 = [sp.tile([128, BC], BF16, tag=f"s2_{m}", name=f"s2_{m}_{t}") for m in range(MT_H)]
                s3 = sp.tile([A, BC], BF16, tag="s3", name=f"s3_{t}")
                sf = [
                    [sp.tile([128, BC], F32, tag=f"sf{l}_{m}", name=f"sf{l}_{m}_{t}") for m in range(MT_H)]
                    for l in range(2)
                ]
                sf3 = sp.tile([A, BC], F32, tag="sf3", name=f"sf3_{t}")
                for c in range(NCHUNK):
                    cs = slice(c * CHUNK, (c + 1) * CHUNK)
                    # ----- layer 1 (no matmul: P1 is the constant input) -----
                    for m in range(MT_H):
                        # v' = u*decay + P1
                        nc.vector.scalar_tensor_tensor(
                            u1[m][:, cs], u1[m][:, cs], DECAY, P1[m][:, cs],
                            AOP.mult, AOP.add,
                        )
                        # s = (v' >= 1) -> fp32 on GpSimd, bf16 via ACT copy
                        nc.gpsimd.tensor_scalar(
                            sf[0][m][:, cs], u1[m][:, cs], THRESHOLD, None, AOP.is_ge
                        )
                        nc.scalar.activation(
                            s1[m][:, cs], sf[0][m][:, cs], AF.Copy, scale=1.0
                        )
                        # u = (v' < 1) * v'
                        nc.vector.scalar_tensor_tensor(
                            u1[m][:, cs], u1[m][:, cs], THRESHOLD, u1[m][:, cs],
                            AOP.is_lt, AOP.mult,
                        )
                    # ----- layer 2 -----
                    for m in range(MT_H):
                        pv = psyn.tile([128, CHUNK], F32, tag="pv", name=f"pv2_{t}_{c}_{m}")
                        first = True
                        for s in range(NSPLIT):
                            for k in range(MT_H):
                                nc.tensor.matmul(
                                    pv[:], w2[s][k][m][:], s1[k][:, cs],
                                    start=first, stop=(s == NSPLIT - 1 and k == MT_H - 1),
                                )
                                first = False
                        nc.vector.scalar_tensor_tensor(
                            u2[m][:, cs], u2[m][:, cs], DECAY, pv[:],
                            AOP.mult, AOP.add,
                        )
                        nc.gpsimd.tensor_scalar(
                            sf[1][m][:, cs], u2[m][:, cs], THRESHOLD, None, AOP.is_ge
                        )
                        nc.scalar.activation(
                            s2[m][:, cs], sf[1][m][:, cs], AF.Copy, scale=1.0
                        )
                        nc.vector.scalar_tensor_tensor(
                            u2[m][:, cs], u2[m][:, cs], THRESHOLD, u2[m][:, cs],
                            AOP.is_lt, AOP.mult,
                        )
                    # ----- layer 3 -----
                    pv = psyn3.tile([A, CHUNK], F32, tag="pv3", name=f"pv3_{t}_{c}")
                    first = True
                    for s in range(NSPLIT):
                        for k in range(MT_H):
                            nc.tensor.matmul(
                                pv[:], w3[s][k][:], s2[k][:, cs],
                                start=first, stop=(s == NSPLIT - 1 and k == MT_H - 1),
                            )
                            first = False
                    nc.vector.scalar_tensor_tensor(
                        u3[:, cs], u3[:, cs], DECAY, pv[:], AOP.mult, AOP.add
                    )
                    nc.gpsimd.tensor_scalar(
                        sf3[:, cs], u3[:, cs], THRESHOLD, None, AOP.is_ge
                    )
                    nc.scalar.activation(s3[:, cs], sf3[:, cs], AF.Copy, scale=1.0)
                    nc.vector.scalar_tensor_tensor(
                        u3[:, cs], u3[:, cs], THRESHOLD, u3[:, cs], AOP.is_lt, AOP.mult
                    )
                    # ----- accumulate spikes (exact integer counts in PSUM) -----
                    nc.tensor.matmul(
                        acc[:, cs], eyeA[:], s3[:, cs],
                        start=(t == 0), stop=(t == n_steps - 1),
                        skip_group_check=True,
                    )

            # ---------- write out ----------
            out_sb = pre.tile([A, BC], F32, name="out_sb")
            nc.scalar.activation(out_sb[:], acc[:], AF.Copy, scale=1.0)
            nc.gpsimd.dma_start(out=acc_d[:, :], in_=out_sb[:])

    _split_multiwaits(nc)
    return nc


_PROG_CACHE = {}


def _get_program(n_steps=SIM_TIME):
    if n_steps not in _PROG_CACHE:
        _PROG_CACHE[n_steps] = _build_program(n_steps)
    return _PROG_CACHE[n_steps]


def kernel(x, enc_W, enc_b, W1, W2, W3, n_steps=SIM_TIME):
    x = np.asarray(x, np.float32)
    enc_W = np.asarray(enc_W, np.float32)
    enc_b = np.asarray(enc_b, np.float32)
    W1 = np.asarray(W1, np.float32)
    W2 = np.asarray(W2, np.float32)
    W3 = np.asarray(W3, np.float32)

    nc = _get_program(n_steps)

    xT = np.ascontiguousarray(x.T)                       # [D, B]
    encWt = np.ascontiguousarray(enc_W.T)                # lhsT for enc matmul
    encb = np.ascontiguousarray(enc_b.reshape(D, 1))
    omd = np.float32(ONE_MINUS_DECAY)
    w2s = np.stack(_bf16_split(W2.astype(np.float32) * omd))   # [3, H, H] bf16
    w3s = np.stack(_bf16_split(W3.astype(np.float32) * omd))   # [3, H, A] bf16
    eyeA = np.eye(A, dtype=ml_dtypes.bfloat16)

    in_maps = []
    for c in range(N_CORES):
        in_maps.append(
            {
                "xT": np.ascontiguousarray(xT[:, c * BC : (c + 1) * BC]),
                "encWt": encWt,
                "encb": encb,
                "w1": W1,
                "w2s": w2s,
                "w3s": w3s,
                "eyeA": eyeA,
            }
        )

    res = run_bass_kernel_spmd(nc, in_maps, list(range(N_CORES)))
    outs = []
    for c in range(N_CORES):
        acc = res.results[c]["acc"]            # [A, BC] spike counts
        outs.append((acc / np.float32(n_steps)).T)  # [BC, A]
    return np.ascontiguousarray(np.concatenate(outs, axis=0).astype(np.float32))


# revision 11
# speedup vs baseline: 1.1337x; 1.1337x over previous
"""Trainium2 Bass kernel for the NeuromorphicNetwork (LIF spiking net) problem.

Computation (see problem reference):
  rates = sigmoid(x @ enc_W.T + enc_b)            [B, D]
  100 steps of 3-layer LIF:
    v_L = v_L*decay + (in_L @ W_L)*(1-decay);  s_L = v_L >= 1;  v_L = v_L*(v_L<1)
  out = (sum_t s_3) / 100                          [B, A]

Strategy:
  - Data-parallel over batch: 8192 rows -> 8 cores x 1024.
  - Feature-major on-chip layout: activations stored [features, batch] so the
    recurrent matmuls need no transposes (weights stationary, batch streams).
  - rates@W1 is constant across the scan: precomputed once on-device (fp32
    matmuls), so layer 1 needs no per-step matmul at all.
  - Layer 2/3 synaptic matmuls: weights pre-split on host into 3 bf16
    components (hi/lo/lo2, ~24 mantissa bits total). Spikes are {0,1} which
    is exact in bf16, so each product is exact and the fp32 PSUM sum matches
    an fp32 matmul to ~1 ulp -- at 1 bf16 cycle/column instead of 4 for fp32.
  - LIF pointwise update per tile is 3 fused ops:
      v' = (u * decay) + syn       (scalar_tensor_tensor, reads PSUM)
      s  = (v' >= 1)               (tensor_scalar -> bf16 {0,1})
      u  = (v' < 1) * v'           (scalar_tensor_tensor, in-place)
    split across the Vector and GpSimd engines.
  - Output spike accumulation: identity-matmul of s3 into a persistent PSUM
    bank (exact integer counts); /100 happens on host in fp32 (bit-identical
    to the reference's acc/SIM_TIME).
"""

import numpy as np
import ml_dtypes

import bass_rust
import concourse.bass as bass
import concourse.tile as tile
from concourse import mybir
from concourse.bass_utils import run_bass_kernel_spmd

F32 = mybir.dt.float32
BF16 = mybir.dt.bfloat16
AOP = mybir.AluOpType
AF = mybir.ActivationFunctionType

TAU = 20.0
THRESHOLD = 1.0
DT = 1.0
SIM_TIME = 100
B, D, H, A = 8192, 256, 256, 64
N_CORES = 8
BC = B // N_CORES          # batch per core (1024)
CHUNK = 512                # batch chunk for pipelining
NCHUNK = BC // CHUNK
DECAY = float(np.exp(np.float32(-DT / TAU)).astype(np.float32))
ONE_MINUS_DECAY = float(np.float32(1.0) - np.float32(DECAY))
NSPLIT = 3                 # bf16 weight splits


def _split_multiwaits(nc):
    """walrus in this toolchain crashes on instructions carrying >=2 sem
    waits; split extras onto single-wait NoOps just before the instruction."""
    f = nc.m.functions[0]
    for bb in f.blocks:
        insts = list(bb.instructions)
        changed = False
        new_list = []
        for inst in insts:
            si = inst.sync_info
            if si is not None and si.on_wait is not None and len(si.on_wait) > 1:
                waits = list(si.on_wait)
                for w in waits[:-1]:
                    nop = mybir.InstNoOp(name=f"{inst.name}-sw-{w.id}", ins=[], outs=[])
                    nop.engine = inst.engine
                    nop.sync_info = bass_rust.SyncInfo(on_wait=[w], on_update=[])
                    new_list.append(nop)
                inst.sync_info = bass_rust.SyncInfo(
                    on_wait=[waits[-1]], on_update=list(si.on_update or [])
                )
                changed = True
            new_list.append(inst)
        if changed:
            bb.instructions = new_list


def _bf16_split(w, n=NSPLIT):
    """Split fp32 array into n bf16 arrays summing to ~w (24+ bits for n=3)."""
    parts = []
    resid = w.astype(np.float32)
    for _ in range(n):
        p = resid.astype(ml_dtypes.bfloat16)
        parts.append(p)
        resid = resid - p.astype(np.float32)
    return parts


def _build_program(n_steps):
    nc = bass.Bass(trn_type="TRN2", target_bir_lowering=False, debug=False)

    # ---- DRAM I/O (per core) ----
    xT_d = nc.dram_tensor("xT", [D, BC], F32, kind="ExternalInput").ap()
    encWt_d = nc.dram_tensor("encWt", [D, D], F32, kind="ExternalInput").ap()
    encb_d = nc.dram_tensor("encb", [D, 1], F32, kind="ExternalInput").ap()
    w1_d = nc.dram_tensor("w1", [D, H], F32, kind="ExternalInput").ap()
    w2s_d = nc.dram_tensor("w2s", [NSPLIT, H, H], BF16, kind="ExternalInput").ap()
    w3s_d = nc.dram_tensor("w3s", [NSPLIT, H, A], BF16, kind="ExternalInput").ap()
    eyeA_d = nc.dram_tensor("eyeA", [A, A], BF16, kind="ExternalInput").ap()
    acc_d = nc.dram_tensor("acc", [A, BC], F32, kind="ExternalOutput").ap()

    KT_D = D // 128  # k-tiles over 256 features
    MT_H = H // 128  # m-tiles of hidden

    with tile.TileContext(nc) as tc:
        with (
            tc.tile_pool(name="persist", bufs=1) as pp,
            tc.tile_pool(name="spikes", bufs=3) as sp,
            tc.tile_pool(name="pre", bufs=1) as pre,
            tc.tile_pool(name="psyn", bufs=4, space="PSUM") as psyn,
            tc.tile_pool(name="psyn3", bufs=2, space="PSUM") as psyn3,
            tc.tile_pool(name="pacc", bufs=1, space="PSUM") as pacc,
        ):
            # ---------- one-time: load + precompute ----------
            xT = [pre.tile([128, BC], F32, name=f"xT{k}") for k in range(KT_D)]
            encW = [
                [pre.tile([128, 128], F32, name=f"encW{k}{m}") for m in range(KT_D)]
                for k in range(KT_D)
            ]
            encb = [pre.tile([128, 1], F32, name=f"encb{m}") for m in range(MT_H)]
            w1 = [
                [pre.tile([128, 128], F32, name=f"w1{k}{m}") for m in range(MT_H)]
                for k in range(KT_D)
            ]
            for k in range(KT_D):
                nc.gpsimd.dma_start(out=xT[k][:], in_=xT_d[128 * k : 128 * (k + 1), :])
                for m in range(KT_D):
                    nc.gpsimd.dma_start(
                        out=encW[k][m][:],
                        in_=encWt_d[128 * k : 128 * (k + 1), 128 * m : 128 * (m + 1)],
                    )
                    nc.gpsimd.dma_start(
                        out=w1[k][m][:],
                        in_=w1_d[128 * k : 128 * (k + 1), 128 * m : 128 * (m + 1)],
                    )
            for m in range(MT_H):
                nc.gpsimd.dma_start(out=encb[m][:], in_=encb_d[128 * m : 128 * (m + 1), :])

            # weights for layers 2/3 (bf16 splits) + identity for acc
            w2 = [
                [
                    [pre.tile([128, 128], BF16, name=f"w2_{s}_{k}{m}") for m in range(MT_H)]
                    for k in range(MT_H)
                ]
                for s in range(NSPLIT)
            ]
            w3 = [
                [pre.tile([128, A], BF16, name=f"w3_{s}_{k}") for k in range(MT_H)]
                for s in range(NSPLIT)
            ]
            eyeA = pre.tile([128, A], BF16, name="eyeA")
            nc.gpsimd.dma_start(out=eyeA[0:A, :], in_=eyeA_d[:, :])
            nc.gpsimd.dma_start(out=eyeA[A : 2 * A, :], in_=eyeA_d[:, :])
            for s in range(NSPLIT):
                for k in range(MT_H):
                    nc.gpsimd.dma_start(
                        out=w3[s][k][:], in_=w3s_d[s, 128 * k : 128 * (k + 1), :]
                    )
                    for m in range(MT_H):
                        nc.gpsimd.dma_start(
                            out=w2[s][k][m][:],
                            in_=w2s_d[s, 128 * k : 128 * (k + 1), 128 * m : 128 * (m + 1)],
                        )

            # rates = sigmoid(encW @ xT + b); P1 = (rates @ W1) * (1-decay)
            rates = [pre.tile([128, BC], F32, name=f"rates{m}") for m in range(MT_H)]
            P1 = [pre.tile([128, BC], F32, name=f"P1{m}") for m in range(MT_H)]
            for m in range(MT_H):
                for c in range(NCHUNK):
                    pt = psyn.tile([128, CHUNK], F32, tag="pv", name=f"pt_{m}_{c}")
                    for k in range(KT_D):
                        nc.tensor.matmul(
                            pt[:],
                            encW[k][m][:],
                            xT[k][:, c * CHUNK : (c + 1) * CHUNK],
                            start=(k == 0),
                            stop=(k == KT_D - 1),
                        )
                    nc.scalar.activation(
                        rates[m][:, c * CHUNK : (c + 1) * CHUNK],
                        pt[:],
                        AF.Sigmoid,
                        bias=encb[m][:],
                        scale=1.0,
                    )
            for m in range(MT_H):
                for c in range(NCHUNK):
                    pt = psyn.tile([128, CHUNK], F32, tag="pv", name=f"pt_{m}_{c}")
                    for k in range(KT_D):
                        nc.tensor.matmul(
                            pt[:],
                            w1[k][m][:],
                            rates[k][:, c * CHUNK : (c + 1) * CHUNK],
                            start=(k == 0),
                            stop=(k == KT_D - 1),
                        )
                    # P1 = psum * (1-decay), matching reference rounding
                    nc.scalar.activation(
                        P1[m][:, c * CHUNK : (c + 1) * CHUNK],
                        pt[:],
                        AF.Copy,
                        scale=ONE_MINUS_DECAY,
                    )

            # ---------- persistent state ----------
            u1 = [pp.tile([128, BC], F32, name=f"u1_{m}") for m in range(MT_H)]
            u2 = [pp.tile([128, BC], F32, name=f"u2_{m}") for m in range(MT_H)]
            u3 = pp.tile([128, CHUNK], F32, name="u3")
            for m in range(MT_H):
                nc.vector.memset(u1[m][:], 0.0)
                nc.vector.memset(u2[m][:], 0.0)
            nc.vector.memset(u3[:], 0.0)
            acc = pacc.tile([A, BC], F32, name="accp")

            # ---------- the 100-step scan ----------
            for t in range(n_steps):
                s1 = [sp.tile([128, BC], BF16, tag=f"s1_{m}", name=f"s1_{m}_{t}") for m in range(MT_H)]
                s2 = [sp.tile([128, BC], BF16, tag=f"s2_{m}", name=f"s2_{m}_{t}") for m in range(MT_H)]
                s3 = sp.tile([128, CHUNK], BF16, tag="s3", name=f"s3_{t}")
                sf = [
                    [sp.tile([128, BC], F32, tag=f"sf{l}_{m}", name=f"sf{l}_{m}_{t}") for m in range(MT_H)]
                    for l in range(2)
                ]
                sf3 = sp.tile([128, CHUNK], F32, tag="sf3", name=f"sf3_{t}")
                vv1 = [sp.tile([128, BC], F32, tag=f"vv1_{m}", name=f"vv1_{m}_{t}") for m in range(MT_H)]
                vv2 = [
                    [sp.tile([128, CHUNK], F32, tag=f"vv2_{m}{c}", name=f"vv2_{m}{c}_{t}") for c in range(NCHUNK)]
                    for m in range(MT_H)
                ]
                vv3 = sp.tile([128, CHUNK], F32, tag="vv3", name=f"vv3_{t}")
                # ----- layer 1 (no matmul; full-width ops) -----
                for m in range(MT_H):
                    # v' = u*decay + P1  (staged into vv1 so the state loop
                    # stays DVE-only and the spike path runs in parallel)
                    nc.vector.scalar_tensor_tensor(
                        vv1[m][:], u1[m][:], DECAY, P1[m][:], AOP.mult, AOP.add
                    )
                    # u = (v' < 1) * v'
                    nc.vector.scalar_tensor_tensor(
                        u1[m][:], vv1[m][:], THRESHOLD, vv1[m][:], AOP.is_lt, AOP.mult
                    )
                    # s = (v' >= 1) -> fp32 on GpSimd, bf16 via ACT copy
                    nc.gpsimd.tensor_scalar(
                        sf[0][m][:], vv1[m][:], THRESHOLD, None, AOP.is_ge
                    )
                    nc.scalar.activation(s1[m][:], sf[0][m][:], AF.Copy, scale=1.0)
                for c in range(NCHUNK):
                    cs = slice(c * CHUNK, (c + 1) * CHUNK)
                    # ----- layer 2 -----
                    for m in range(MT_H):
                        pv = psyn.tile([128, CHUNK], F32, tag="pv", name=f"pv2_{t}_{c}_{m}")
                        first = True
                        for s in range(NSPLIT):
                            for k in range(MT_H):
                                nc.tensor.matmul(
                                    pv[:], w2[s][k][m][:], s1[k][:, cs],
                                    start=first, stop=(s == NSPLIT - 1 and k == MT_H - 1),
                                )
                                first = False
                        nc.vector.scalar_tensor_tensor(
                            vv2[m][c][:], u2[m][:, cs], DECAY, pv[:],
                            AOP.mult, AOP.add,
                        )
                        nc.vector.scalar_tensor_tensor(
                            u2[m][:, cs], vv2[m][c][:], THRESHOLD, vv2[m][c][:],
                            AOP.is_lt, AOP.mult,
                        )
                        nc.gpsimd.tensor_scalar(
                            sf[1][m][:, cs], vv2[m][c][:], THRESHOLD, None, AOP.is_ge
                        )
                        nc.scalar.activation(
                            s2[m][:, cs], sf[1][m][:, cs], AF.Copy, scale=1.0
                        )
                    # ----- layer 3 matmul: chunk c -> partitions 64c..64c+63 -----
                    if c == 0:
                        pv3 = psyn3.tile([128, CHUNK], F32, tag="pv3", name=f"pv3_{t}")
                    ps = slice(A * c, A * (c + 1))
                    first = True
                    for s in range(NSPLIT):
                        for k in range(MT_H):
                            nc.tensor.matmul(
                                pv3[ps, :], w3[s][k][:], s2[k][:, cs],
                                start=first, stop=(s == NSPLIT - 1 and k == MT_H - 1),
                                skip_group_check=True,
                            )
                            first = False
                # ----- layer 3 pointwise: both chunks at once [128, CHUNK] -----
                nc.vector.scalar_tensor_tensor(
                    vv3[:], u3[:], DECAY, pv3[:], AOP.mult, AOP.add
                )
                nc.vector.scalar_tensor_tensor(
                    u3[:], vv3[:], THRESHOLD, vv3[:], AOP.is_lt, AOP.mult
                )
                nc.gpsimd.tensor_scalar(sf3[:], vv3[:], THRESHOLD, None, AOP.is_ge)
                nc.scalar.activation(s3[:], sf3[:], AF.Copy, scale=1.0)
                # ----- accumulate spikes (exact integer counts in PSUM) -----
                for c in range(NCHUNK):
                    cs = slice(c * CHUNK, (c + 1) * CHUNK)
                    ps = slice(A * c, A * (c + 1))
                    nc.tensor.matmul(
                        acc[:, cs], eyeA[ps, :], s3[ps, :],
                        start=(t == 0), stop=(t == n_steps - 1),
                        skip_group_check=True,
                    )

            # ---------- write out ----------
            out_sb = pre.tile([A, BC], F32, name="out_sb")
            nc.scalar.activation(out_sb[:], acc[:], AF.Copy, scale=1.0)
            nc.gpsimd.dma_start(out=acc_d[:, :], in_=out_sb[:])

    _split_multiwaits(nc)
    return nc


_PROG_CACHE = {}


def _get_program(n_steps=SIM_TIME):
    if n_steps not in _PROG_CACHE:
        _PROG_CACHE[n_steps] = _build_program(n_steps)
    return _PROG_CACHE[n_steps]


def kernel(x, enc_W, enc_b, W1, W2, W3, n_steps=SIM_TIME):
    x = np.asarray(x, np.float32)
    enc_W = np.asarray(enc_W, np.float32)
    enc_b = np.asarray(enc_b, np.float32)
    W1 = np.asarray(W1, np.float32)
    W2 = np.asarray(W2, np.float32)
    W3 = np.asarray(W3, np.float32)

    nc = _get_program(n_steps)

    xT = np.ascontiguousarray(x.T)                       # [D, B]
    encWt = np.ascontiguousarray(enc_W.T)                # lhsT for enc matmul
    encb = np.ascontiguousarray(enc_b.reshape(D, 1))
    omd = np.float32(ONE_MINUS_DECAY)
    w2s = np.stack(_bf16_split(W2.astype(np.float32) * omd))   # [3, H, H] bf16
    w3s = np.stack(_bf16_split(W3.astype(np.float32) * omd))   # [3, H, A] bf16
    eyeA = np.eye(A, dtype=ml_dtypes.bfloat16)

    in_maps = []
    for c in range(N_CORES):
        in_maps.append(
            {
                "xT": np.ascontiguousarray(xT[:, c * BC : (c + 1) * BC]),
                "encWt": encWt,
                "encb": encb,
                "w1": W1,
                "w2s": w2s,
                "w3s": w3s,
                "eyeA": eyeA,
            }
        )

    res = run_bass_kernel_spmd(nc, in_maps, list(range(N_CORES)))
    outs = []
    for c in range(N_CORES):
        acc = res.results[c]["acc"]            # [A, BC] spike counts
        outs.append((acc / np.float32(n_steps)).T)  # [BC, A]
    return np.ascontiguousarray(np.concatenate(outs, axis=0).astype(np.float32))
